# revision 39
# baseline (speedup 1.0000x reference)
"""Trainium2 Bass kernel for nn_HadaMard: fused proj + 2xLayerNorm + outer product.

Reference computation (per batch b, spatial index n in 0..N):
  proj[n, d] = sum_c q[b, c, n] * Wp[d, c] + bp[d]        # [N, 256]
  qn = LN_over_d(proj) * g1 + b1                          # [N, 256]
  xn = LN_over_e(x[b, :, n]) * g2 + b2                    # [N, 32]
  out[b, d*32+e, n] = qn[n, d] * xn[n, e]                 # [8192, N]

Sharding: data-parallel over B=8, one batch per NeuronCore.

Transposed on-chip layout: spatial n on partitions (8 tiles of 128, n = 128j+p),
channels on the free axis.
  - proj^T via PE: lhsT = q column-slice (natural layout), rhs = Wp^T chunk;
    j-major matmul order so tile 0 finishes (and the mul pipeline starts) as
    soon as the last input chunk lands; 8 PSUM banks, one per n-tile.
  - x-side LayerNorm runs first (needs only the input pack) to fill the
    load/matmul window; q-side stats per tile via bn_stats/bn_aggr (one DVE
    pass gives mean+var); sqrt on Act; reciprocal on DVE ([128,1] ops are
    latency-only); normalize to bf16 via Act Identity (scale=rstd,
    bias=-mean*rstd).
  - outer product: per n-tile, 32 scalar-broadcast multiplies
    out[:, e*256:(e+1)*256] = qn * xn[:, e].  The 256 multiplies are split
    across DVE (4x bf16 mode, ~127ns), Pool (~213ns), and Act (Copy with
    per-partition scale, ~398ns) to balance engine load (HM_PAT tunable,
    default 16/11/5 per tile).
  - all output lives in one SBUF megatile [128, 64Ki] bf16; stored with 16
    column-chunk DMAs into outT [1024, 8192] bf16 whose row r = 8p + j
    (n-interleaved) so each store enumerates all 1024 DRAM rows in its
    leading dimension; the host de-interleaves, transposes to [CD, H, W] and
    casts to f32.

Inputs are packed host-side into one [1024, 1312] bf16 tensor per core
(Wp^T | x^T | q share the 1024-row structure), loaded as 24 column-piece DMAs
over the SP/Pool queues: the first piece of every chunk carries Wp^T + x^T +
tile 0's q columns so tile 0's contract chain (and hence the whole mul
pipeline) starts as early as possible.
"""

import numpy as np

_CACHE = {}

B, C1, H, W = 8, 1024, 32, 32
C2 = 32
Cp = 256
N = 1024  # H*W
CD = Cp * C2  # 8192
EPS = 1e-5
NT = 8  # n-tiles of 128 partitions
KC = 8  # contract chunks of 128
PACKW = C1 + Cp + C2  # 1312 packed input columns: wpt | xT | q
QOFF = Cp + C2  # q starts after wpt and xT
NCHUNK = 16  # output column chunks (overridable via HM_NCHUNK)


def _mul_pattern():
    import os

    pat = os.environ.get("HM_PAT", "")
    if len(pat) == C2 and set(pat) <= set("DPA"):
        return pat
    # default split per tile: 16 DVE, 11 Pool, 5 Act, interleaved
    counts = {"D": 16, "P": 11, "A": 5}
    out = []
    acc = {"D": 0.0, "P": 0.0, "A": 0.0}
    for _ in range(C2):
        for k in counts:
            acc[k] += counts[k] / C2
        pick = max(acc, key=lambda k: acc[k])
        acc[pick] -= 1.0
        out.append(pick)
    return "".join(out)


def _build_nc(simple):
    import os

    import concourse.bacc as bacc
    import concourse.bass as bass
    import concourse.mybir as mybir
    import concourse.tile as tile

    F32 = mybir.dt.float32
    BF16 = mybir.dt.bfloat16
    MULT = mybir.AluOpType.mult
    ADD = mybir.AluOpType.add
    SUB = mybir.AluOpType.subtract
    SQRT = mybir.ActivationFunctionType.Sqrt
    COPY = mybir.ActivationFunctionType.Copy
    IDENT = mybir.ActivationFunctionType.Identity

    qn_on_act = os.environ.get("HM_QN", "act") == "act"
    qn_dve_js = {int(v) for v in os.environ.get("HM_QNDVEJS", "").split(",") if v}
    nchunk = int(os.environ.get("HM_NCHUNK", str(NCHUNK)))
    epc = C2 // nchunk
    xn_on_act = os.environ.get("HM_XN", "dve") == "act"
    newton = os.environ.get("HM_RSQRT", "act") == "newton"
    pat = _mul_pattern()
    swaps = {}
    for item in os.environ.get("HM_SWAPS", "").split(";"):
        if item:
            e_, j_, g_ = item.split(":")
            swaps[(int(e_), int(j_))] = g_
    I32 = mybir.dt.int32
    SHR = mybir.AluOpType.logical_shift_right

    nc = bacc.Bacc(None, target_bir_lowering=False)

    pack_d = nc.dram_tensor("pack", [C1, PACKW], BF16, kind="ExternalInput")
    if not simple:
        g1r_d = nc.dram_tensor("g1r", [128, Cp], BF16, kind="ExternalInput")
        b1r_d = nc.dram_tensor("b1r", [128, Cp], BF16, kind="ExternalInput")
        g2r_d = nc.dram_tensor("g2r", [128, C2], F32, kind="ExternalInput")
        b2r_d = nc.dram_tensor("b2r", [128, C2], F32, kind="ExternalInput")
        bpr_d = nc.dram_tensor("bpr", [1, Cp], BF16, kind="ExternalInput")
    out_d = nc.dram_tensor("outT", [N, CD], BF16, kind="ExternalOutput")

    with tile.TileContext(nc) as tc:
        with (
            tc.tile_pool(name="inp", bufs=1) as inp,
            tc.tile_pool(name="sml", bufs=1) as sml,
            tc.tile_pool(name="meg", bufs=1) as megp,
            tc.tile_pool(name="ps", bufs=1, space=bass.MemorySpace.PSUM) as ps,
        ):
            eps_t = sml.tile([128, 1], F32, tag="eps")
            nc.vector.memset(eps_t[:], EPS)
            if newton:
                magic = sml.tile([128, 1], I32, tag="magic")
                nc.vector.memset(magic[:], 0x5F3759DF)
            else:
                # dummy sqrt so the act-table pass loads the sqrt set (which
                # also covers Copy/Identity) exactly once
                warm = sml.tile([128, 1], F32, tag="warm")
                nc.scalar.activation(warm[:], eps_t[:], SQRT, bias=eps_t[:])

            nit = int(os.environ.get("HM_NIT", "1"))
            rseng = nc.gpsimd if os.environ.get("HM_RSENG", "dve") == "pool" else nc.vector

            def emit_rsqrt(rst, var_ap, pre):
                """rst = 1/sqrt(var + EPS), all free-size-1 ops (zero cost)."""
                if newton:
                    ve = sml.tile([128, 1], F32, tag=f"{pre}ve")
                    rseng.tensor_scalar_add(ve[:], var_ap, EPS)
                    si = sml.tile([128, 1], I32, tag=f"{pre}si")
                    rseng.tensor_scalar(
                        si[:], ve[:].bitcast(I32), 1, None, op0=SHR
                    )
                    y = sml.tile([128, 1], I32, tag=f"{pre}y")
                    rseng.tensor_tensor(y[:], magic[:], si[:], op=SUB)
                    yf = y[:].bitcast(F32)
                    for it in range(nit):
                        a = sml.tile([128, 1], F32, tag=f"{pre}a{it}", name=f"{pre}a{it}")
                        rseng.tensor_tensor(a[:], ve[:], yf, op=MULT)
                        rseng.tensor_tensor(a[:], a[:], yf, op=MULT)
                        rseng.tensor_scalar(
                            a[:], a[:], -0.5, 1.5, op0=MULT, op1=ADD
                        )
                        dst = rst if it == nit - 1 else sml.tile(
                            [128, 1], F32, tag=f"{pre}y{it}", name=f"{pre}y{it}"
                        )
                        rseng.tensor_tensor(dst[:], yf, a[:], op=MULT)
                        yf = dst[:]
                else:
                    sd = sml.tile([128, 1], F32, tag=f"{pre}sd")
                    nc.scalar.activation(sd[:], var_ap, SQRT, bias=eps_t[:])
                    scr = sml.tile([128, 1], F32, tag=f"{pre}scr")
                    nc.vector.reciprocal_approx_accurate(rst[:], sd[:], scr[:])

            qmap = {"S": nc.sync, "P": nc.gpsimd, "A": nc.scalar}
            in_order = os.environ.get("HM_INQ", "SPSPSPSPPSPSPSPSSSSSSSSS")
            insplit = os.environ.get("HM_INSPLIT", "c416,864")
            if insplit.startswith("c"):
                bounds = [int(v) for v in insplit[1:].split(",")]
                bounds = [0] + bounds + [PACKW]
            else:
                n_ = int(insplit)
                w = PACKW // n_
                bounds = [w * i for i in range(n_)] + [PACKW]
            pk = [inp.tile([128, PACKW], BF16, tag=f"pk{k}", name=f"pk{k}")
                  for k in range(KC)]
            qi = 0
            for s_ in range(len(bounds) - 1):
                cs = slice(bounds[s_], bounds[s_ + 1])
                for k in range(KC):
                    qmap[in_order[qi % len(in_order)]].dma_start(
                        pk[k][:, cs], pack_d[128 * k : 128 * (k + 1), cs]
                    )
                    qi += 1

            if not simple:
                g1r = sml.tile([128, Cp], BF16, tag="g1r")
                nc.scalar.dma_start(g1r[:], g1r_d[:])
                b1r = sml.tile([128, Cp], BF16, tag="b1r")
                nc.scalar.dma_start(b1r[:], b1r_d[:])
                g2r = sml.tile([128, C2], F32, tag="g2r")
                nc.scalar.dma_start(g2r[:], g2r_d[:])
                b2r = sml.tile([128, C2], F32, tag="b2r")
                nc.scalar.dma_start(b2r[:], b2r_d[:])
                bpr = sml.tile([1, Cp], BF16, tag="bpr")
                nc.scalar.dma_start(bpr[:], bpr_d[:])
                ones1 = sml.tile([1, 128], BF16, tag="ones1")
                nc.vector.memset(ones1[:], 1.0)

            meg = megp.tile([128, NT * CD], BF16, tag="meg")

            nwarm = int(os.environ.get("HM_PEWARM", "0"))

            # ---- x-side LayerNorm first: only needs the pack chunk, so it
            # runs during the load/matmul window
            xn = []
            for j in range(NT):
                xr = pk[j][:, Cp : Cp + C2]
                xs6 = sml.tile([128, 6], F32, tag=f"xs6_{j}")
                nc.vector.bn_stats(xs6[:], xr)
                xmv = sml.tile([128, 2], F32, tag=f"xmv_{j}")
                nc.vector.bn_aggr(xmv[:], xs6[:])
                xrs = sml.tile([128, 1], F32, tag=f"xrs_{j}")
                emit_rsqrt(xrs, xmv[:, 1:2], f"x{j}")
                xj = sml.tile([128, C2], F32, tag=f"xn_{j}")
                if xn_on_act:
                    nbx = sml.tile([128, 1], F32, tag=f"nbx_{j}")
                    nc.gpsimd.tensor_scalar(
                        nbx[:], xmv[:, 0:1], -1.0, xrs[:], op0=MULT, op1=MULT
                    )
                    nc.scalar.activation(
                        xj[:], xr, IDENT, bias=nbx[:], scale=xrs[:]
                    )
                else:
                    nc.vector.tensor_scalar(
                        xj[:], xr, xmv[:, 0:1], xrs[:], op0=SUB, op1=MULT
                    )
                if not simple:
                    nc.vector.tensor_tensor(xj[:], xj[:], g2r[:], op=MULT)
                    nc.vector.tensor_tensor(xj[:], xj[:], b2r[:], op=ADD)
                xn.append(xj)

            # ---- projection: proj^T[j][n, d] accumulated over contract chunks
            pj = [
                ps.tile([128, Cp], F32, tag=f"pj{j}", name=f"pj{j}")
                for j in range(NT)
            ]
            if nwarm:
                wmw = sml.tile([128, 128], BF16, tag="wmw")
                nc.vector.memset(wmw[:], 0.0)
                for _ in range(nwarm):
                    nc.tensor.matmul(
                        pj[NT - 1][:, 0:128], wmw[:], wmw[:], start=True, stop=True
                    )
            if os.environ.get("HM_JMAJOR", "1") == "1":
                mm_order = [(k, j) for j in range(NT) for k in range(KC)]
            else:
                mm_order = [(k, j) for k in range(KC) for j in range(NT)]
            for k, j in mm_order:
                nc.tensor.matmul(
                    pj[j][:],
                    pk[k][:, QOFF + 128 * j : QOFF + 128 * (j + 1)],
                    pk[k][:, 0:Cp],
                    start=(k == 0),
                    stop=(k == KC - 1 and simple),
                )
            if not simple:
                for j in range(NT):
                    nc.tensor.matmul(
                        pj[j][:], ones1[:], bpr[:], start=False, stop=True
                    )

            # ---- per n-tile: q-side LN stats + normalize
            qn = []
            for j in range(NT):
                st6 = sml.tile([128, 6], F32, tag=f"st6_{j}")
                nc.vector.bn_stats(st6[:], pj[j][:])
                mv = sml.tile([128, 2], F32, tag=f"mv_{j}")
                nc.vector.bn_aggr(mv[:], st6[:])
                rst = sml.tile([128, 1], F32, tag=f"rst_{j}")
                emit_rsqrt(rst, mv[:, 1:2], f"q{j}")
                qj = sml.tile([128, Cp], BF16, tag=f"qn_{j}")
                if qn_on_act and j not in qn_dve_js:
                    nb = sml.tile([128, 1], F32, tag=f"nb_{j}")
                    nc.gpsimd.tensor_scalar(
                        nb[:], mv[:, 0:1], -1.0, rst[:], op0=MULT, op1=MULT
                    )
                    nc.scalar.activation(
                        qj[:], pj[j][:], IDENT, bias=nb[:], scale=rst[:]
                    )
                else:
                    nc.vector.tensor_scalar(
                        qj[:], pj[j][:], mv[:, 0:1], rst[:], op0=SUB, op1=MULT
                    )
                if not simple:
                    nc.vector.tensor_tensor(qj[:], qj[:], g1r[:], op=MULT)
                    nc.vector.tensor_tensor(qj[:], qj[:], b1r[:], op=ADD)
                qn.append(qj)

            # ---- outer product into megatile + chunked store
            # meg free offset(j, e) = (e//epc)*NT*epc*Cp + j*epc*Cp + (e%epc)*Cp
            chunk_w = NT * epc * Cp
            ei_inner = os.environ.get("HM_EIINNER", "1") == "1"
            for c in range(nchunk):
                if ei_inner:
                    jei = [(j, ei) for j in range(NT) for ei in range(epc)]
                else:
                    jei = [(j, ei) for ei in range(epc) for j in range(NT)]
                for j, ei in jei:
                    if True:
                        e = epc * c + ei
                        off = c * chunk_w + j * epc * Cp + ei * Cp
                        dst = meg[:, off : off + Cp]
                        eng = pat[e]
                        if (e, j) in swaps:
                            eng = swaps[(e, j)]
                        if eng == "A":
                            nc.scalar.activation(
                                dst, qn[j][:], COPY, bias=0.0,
                                scale=xn[j][:, e : e + 1],
                            )
                        elif eng == "P":
                            nc.gpsimd.tensor_scalar_mul(
                                dst, qn[j][:], xn[j][:, e : e + 1]
                            )
                        else:
                            nc.vector.tensor_scalar_mul(
                                dst, qn[j][:], xn[j][:, e : e + 1]
                            )
                # store chunk c: dst rows r = 8p+j, cols [epc*Cp*c : +epc*Cp)
                outq = os.environ.get("HM_OUTQ", "SSSSSSSS")
                qmap[outq[c % len(outq)]].dma_start(
                    out_d[:, epc * Cp * c : epc * Cp * (c + 1)],
                    meg[:, c * chunk_w : (c + 1) * chunk_w],
                )

    nc.compile()
    return nc


def _is_simple(bp, g1, b1, g2, b2):
    return (
        np.allclose(np.asarray(bp), 0)
        and np.allclose(np.asarray(g1), 1)
        and np.allclose(np.asarray(b1), 0)
        and np.allclose(np.asarray(g2), 1)
        and np.allclose(np.asarray(b2), 0)
    )


def _host_inputs(q, x, Wp, bp, g1, b1, g2, b2):
    """Build the 8 per-core input maps (packed bf16 input per core)."""
    import os

    import ml_dtypes

    bf16 = ml_dtypes.bfloat16
    simple = os.environ.get("HM_SIMPLE", "1") == "1"
    qf = np.asarray(q, dtype=np.float32).reshape(B, C1, N)
    xf = np.asarray(x, dtype=np.float32).reshape(B, C2, N)
    wpt = np.asarray(Wp, dtype=np.float32).T  # [C1, Cp]

    pack = np.empty((B, C1, PACKW), dtype=bf16)
    pack[:, :, :Cp] = wpt.astype(bf16)[None]
    pack[:, :, Cp : Cp + C2] = xf.transpose(0, 2, 1).astype(bf16)
    pack[:, :, Cp + C2 :] = qf.astype(bf16)

    extras = {}
    if not simple:
        extras["g1r"] = np.ascontiguousarray(
            np.broadcast_to(np.asarray(g1, np.float32), (128, Cp))
        ).astype(bf16)
        extras["b1r"] = np.ascontiguousarray(
            np.broadcast_to(np.asarray(b1, np.float32), (128, Cp))
        ).astype(bf16)
        extras["g2r"] = np.ascontiguousarray(
            np.broadcast_to(np.asarray(g2, np.float32), (128, C2))
        )
        extras["b2r"] = np.ascontiguousarray(
            np.broadcast_to(np.asarray(b2, np.float32), (128, C2))
        )
        extras["bpr"] = np.asarray(bp, np.float32).reshape(1, Cp).astype(bf16)

    in_maps = []
    for bb in range(B):
        m = {"pack": np.ascontiguousarray(pack[bb])}
        m.update(extras)
        in_maps.append(m)
    return in_maps


def _run(in_maps, trace=False):
    import os

    from concourse.bass_utils import run_bass_kernel_spmd

    key = "nc" + os.environ.get("HM_SIMPLE", "1")
    if key not in _CACHE:
        _CACHE[key] = _build_nc(os.environ.get("HM_SIMPLE", "1") == "1")
    nc = _CACHE[key]
    res = run_bass_kernel_spmd(nc, in_maps, core_ids=list(range(B)), trace=trace)
    return res


def _decode_out(outT):
    """outT [1024, 8192] bf16 with row r = 8p+j (n = 128j+p), col = e*256+d
    -> out [CD, H, W] f32."""
    o = np.asarray(outT, dtype=np.float32)
    # [p, j, e, d] -> [d, e, j, p]; n iterates (j, p) in row-major order
    return (
        o.reshape(128, NT, C2, Cp)
        .transpose(3, 2, 1, 0)
        .reshape(CD, H, W)
    )


def kernel(q, x, Wp, bp, g1, b1, g2, b2):
    import os

    simple = _is_simple(bp, g1, b1, g2, b2)
    os.environ["HM_SIMPLE"] = "1" if simple else "0"
    in_maps = _host_inputs(q, x, Wp, bp, g1, b1, g2, b2)
    res = _run(in_maps, trace=False)
    out = np.empty((B, CD, H, W), np.float32)
    for bb in range(B):
        out[bb] = _decode_out(res.results[bb]["outT"])
    _CACHE["last_res"] = res
    return out
